# revision 1
# baseline (speedup 1.0000x reference)
"""MoE layer (dense all-experts SwiGLU + router-weighted sum) on 8 TRN2 cores.

Expert-parallel: core e holds expert e's weights (E=8). Every core sees the
full token stream x (shipped pre-transposed as xt [H, N]) and computes
  y_e = softmax(x @ W_router)[:, e] * ((silu(x@Wg_e) * (x@Wu_e)) @ Wd_e)
The host sums the 8 per-expert outputs.

~1377us vs the 1583us fp32r baseline (PE F-cycle floor ~1341us; measured
steady-state matmul cadence is the 216ns minimum = 512 cols @2.4GHz + NX
dispatch). What bought the speedup, in order of impact:
  - all matmul operands in bf16 (same 1 col/cycle PE rate as f32r, ~2e-3
    extra rel err, far under the 2e-2 gate). Halves SBUF+DMA so ALL
    weights (wg/wu/wd = 12MB) are SBUF-resident: the per-block 8MB wd
    re-stream is gone and stage 2 never waits on DMA. bf16 stationary
    weights also get FWL (LDWEIGHTS 97ns, fully hidden -> 216ns/MM vs
    233ns for f32r).
  - silu via the exp-family table: silu(g)*u = 0.5*(p + p*tanh(0.5 g)),
    p = g*u, with the 0.5 folded into wd host-side. The baseline
    alternated Silu/Exp activation tables every block: 32 ACT_TABLE_LOADs
    (~1.3us each) that stalled PSUM eviction at block starts. Tanh and
    Exp share one table set -> 1 load total, and HAM (the PE clock gate)
    stays at 8/8 the whole run.
  - column-packed router: the M=8 logit matmuls run 4-concurrent via
    tile_position col groups (2 rounds + one K=32 sum-matmul = 3 PE slots
    per block instead of 8). Group partials cross partitions via DVE
    eviction + tiny SBUF->SBUF DMAs (DVE is lane-locked; DMA can't read
    PSUM). Router pieces are spread through stage-2's h1 i-loop so the PE
    never waits on the DVE/DMA/ScalarE chain.
  - y evictions split into DVE + ScalarE halves: 4 serialized 739ns DVE
    evictions were gating h1's first matmuls (PSUM slot reuse) ~2.3us per
    block; two engines clear each bank in ~400ns.
  - startup: host pre-arranges weights into exact SBUF layouts (all DMAs
    contiguous), wg/wu stream in i-stripe order, xt is one 1MB descriptor
    per block, and ~72 tiny matmuls on wr warm the HAM clock gate while
    xt(0) streams, so real work starts ~9us in at full clock.
  - ht double-buffered across blocks; xt prefetched 2 blocks ahead.
  - boundary filler: the next block's packed router AND its first G-group
    (no y_ps banks touched) execute at the h0->h1 boundary, absorbing the
    ~300-450ns/link eviction-semaphore latency that stalled h1's first
    matmuls - steady state now shows zero PE stalls above trace noise.

Known residue (~36us over the F-cycle floor): ~7us NEFF prologue, ~12us
BSP epilogue, ~17us NX dispatch tax (216 vs 213.3ns), ~10us block-
boundary sem latencies. Runs sometimes come back ~1670us with identical
(correct) output: the chip drops the whole NeuronCore domain to 2.0GHz
(P0 power state, spacing 259ns = 216*2.4/2.0) - environmental, not
kernel-dependent.
"""
import numpy as np
import ml_dtypes

import concourse.bass as bass
import concourse.mybir as mybir
import concourse.tile as tile
from concourse import bacc
from concourse.bass_utils import run_bass_kernel_spmd

P = 128
H, I, E = 1024, 2048, 8
N = 8192       # tokens = 4 * 2048
HK = H // P    # 8 contraction chunks over H
IK = I // P    # 16 chunks over I
TB = 512       # token block
NB = N // TB   # 16 blocks
NM = TB // P   # 4 token subtiles per block
NH = H // 512  # 2 output column halves
SW = HK * P    # wg/wu stripe width: stripe i holds cols i*SW..(i+1)*SW

F32 = mybir.dt.float32
F32R = mybir.dt.float32r
BF16 = mybir.dt.bfloat16
AF = mybir.ActivationFunctionType
MUL = mybir.AluOpType.mult
ADD = mybir.AluOpType.add

# set by a driver (test.py) to profile; harness path keeps defaults
TRACE = False
LAST_EXEC_NS = None

_CACHE = {}


def _build():
    nc = bacc.Bacc("TRN2", target_bir_lowering=False, debug=False)

    xt_d = nc.dram_tensor("xt", [H, N], BF16, kind="ExternalInput").ap()
    wg_d = nc.dram_tensor("wg", [P, IK * SW], BF16, kind="ExternalInput").ap()
    wu_d = nc.dram_tensor("wu", [P, IK * SW], BF16, kind="ExternalInput").ap()
    wd_d = nc.dram_tensor("wd", [P, IK * H], BF16, kind="ExternalInput").ap()
    wr_d = nc.dram_tensor("wr", [P, HK * E], BF16, kind="ExternalInput").ap()
    sel_d = nc.dram_tensor("sel", [E, 2], F32R, kind="ExternalInput").ap()
    msum_d = nc.dram_tensor("msum", [32, E], F32R, kind="ExternalInput").ap()
    y_d = nc.dram_tensor("y", [N, H], F32, kind="ExternalOutput").ap()

    with tile.TileContext(nc) as tc:
        with (
            tc.tile_pool(name="const", bufs=1) as const,
            tc.tile_pool(name="xtp", bufs=3) as xtp,
            tc.tile_pool(name="htp", bufs=2) as htp,
            tc.tile_pool(name="s1p", bufs=8) as s1p,
            tc.tile_pool(name="evp", bufs=6) as evp,
            tc.tile_pool(name="rtp", bufs=2) as rtp,
            tc.tile_pool(name="wp", bufs=2) as wp,
            tc.tile_pool(name="psgu", bufs=3, space="PSUM") as psgu,
            tc.tile_pool(name="psy", bufs=4, space="PSUM") as psy,
            tc.tile_pool(name="psl", bufs=1, space="PSUM") as psl,
        ):
            # resident weights, in the exact host-prearranged layouts
            wg_sb = const.tile([P, IK * SW], BF16)
            wu_sb = const.tile([P, IK * SW], BF16)
            wd_sb = const.tile([P, IK * H], BF16)
            wr_sb = const.tile([P, HK * E], BF16)
            sel_sb = const.tile([E, 2], F32R)
            msum_sb = const.tile([32, E], F32R)
            # wr first (the HAM warmup matmuls need it; SWDGE's first
            # descriptor fires ~5us late, so these stay on sync), then xt(0);
            # sel/msum are only needed by block 0's mid-stage-1 router hooks
            nc.sync.dma_start(out=wr_sb[:], in_=wr_d[:])

            def load_xt(b):
                # one 1MB descriptor per 512-token block (8 separate chunk
                # DMAs cost ~650ns latency each and stretched startup ~5us)
                tok = slice(b * TB, (b + 1) * TB)
                blk = xtp.tile([P, HK * TB], BF16, tag="xt", name=f"xt{b}")
                nc.sync.dma_start(
                    out=blk[:].rearrange("p (k t) -> p k t", k=HK),
                    in_=xt_d[:, tok].rearrange("(k p) t -> p k t", p=P),
                )
                return blk

            # xt(0) split in two descriptors: the packed router's first round
            # only needs chunks 0-3, so it starts ~1.7us earlier
            xt_next = xtp.tile([P, HK * TB], BF16, tag="xt", name="xt0")
            tok0 = slice(0, TB)
            for half in range(2):
                rows = slice(half * 4 * P, (half + 1) * 4 * P)
                nc.sync.dma_start(
                    out=xt_next[:, half * 4 * TB:(half + 1) * 4 * TB]
                    .rearrange("p (k t) -> p k t", k=4),
                    in_=xt_d[rows, tok0].rearrange("(k p) t -> p k t", p=P),
                )
            # wg/wu interleaved per 256KB i-stripe: g_step(i)/u_step(i) only
            # need stripe i, so the PE starts right after xt(0) lands;
            # sel/msum slot in after stripe 0 (needed only by the i==4 hook)
            for i in range(IK):
                cols = slice(i * SW, (i + 1) * SW)
                nc.sync.dma_start(out=wg_sb[:, cols], in_=wg_d[:, cols])
                nc.sync.dma_start(out=wu_sb[:, cols], in_=wu_d[:, cols])
                if i == 0:
                    nc.sync.dma_start(out=sel_sb[:], in_=sel_d[:])
                    nc.sync.dma_start(out=msum_sb[:], in_=msum_d[:])
            # wd needed only from block-0 stage 2 (~65us in); 4x 1MB
            for q in range(4):
                cols = slice(q * 4 * H, (q + 1) * 4 * H)
                nc.sync.dma_start(out=wd_sb[:, cols], in_=wd_d[:, cols])
            xt_next2 = load_xt(1)

            def router_pack(xt_blk, warm=False):
                """Column-packed router logits: 4 concurrent M=8 matmuls per
                round (col groups 32j), 2 rounds accumulating K=2x512 over
                the 8 H-chunks. 3 PE slots/block instead of 8. The 4 group
                partials land on PSUM partitions 32j..32j+7; DVE evicts them
                lane-aligned, then tiny SBUF->SBUF DMAs compact the groups to
                partitions 0..31 for the K=32 sum-matmul in router_sum (DVE
                cannot move data across partitions; DMA cannot read PSUM)."""
                lt_ps = psl.tile([P, TB], F32, tag="rt", name="lt_ps")
                if warm:
                    # ~72 tiny matmuls on already-resident wr warm the PE's
                    # HAM clock gate (~3.4us of activity) while xt(0) still
                    # streams, so real work starts at 2.4GHz, not 1.2GHz.
                    # They write scratch rows the packed matmuls reset.
                    # (A memset-sourced variant that starts ~2.4us earlier
                    # measured ~1.5us slower overall - kept the wr version.)
                    for _ in range(72):
                        nc.tensor.matmul(
                            lt_ps[0:8, 0:64],
                            (wr_sb[:, 0:8]),
                            (wr_sb[:, 0:64]),
                            start=True,
                            stop=True,
                        )
                for r in range(2):
                    for j in range(4):
                        k = 4 * r + j
                        nc.tensor.matmul(
                            lt_ps[32 * j:32 * j + 8, :],
                            (wr_sb[:, k * E:(k + 1) * E]),
                            (xt_blk[:, k * TB:(k + 1) * TB]),
                            start=(r == 0),
                            stop=(r == 1),
                            tile_position=(0, 32 * j),
                        )
                lts = rtp.tile([P, TB], F32R, tag="lts", name="lts")
                for j in range(4):
                    nc.vector.tensor_copy(
                        out=lts[32 * j:32 * j + 8, :],
                        in_=lt_ps[32 * j:32 * j + 8, :],
                    )
                lt32 = rtp.tile([32, TB], F32R, tag="lt32", name="lt32")
                for j in range(4):
                    nc.gpsimd.dma_start(
                        out=lt32[8 * j:8 * j + 8, :],
                        in_=lts[32 * j:32 * j + 8, :],
                    )
                return lt32

            def router_sum(lt32):
                lt2 = psl.tile([E, TB], F32, tag="rt", name="lt2")
                nc.tensor.matmul(
                    lt2[:], (msum_sb[:]), (lt32[:]), start=True, stop=True
                )
                exp_sb = rtp.tile([E, TB], F32R, tag="exp", name="exp_sb")
                nc.scalar.activation(exp_sb[:], lt2[:], AF.Exp)
                return exp_sb

            def router_weights(exp_sb):
                # w[tok] = exp_e / sum_e' exp_e' via per-subtile transpose-mm.
                # All 4 [denom|numer] matmuls land in one PSUM tile so they
                # issue back-to-back on the PE with no DVE dependency between
                # them (separate bufs=1 tiles would serialize matmul m+1
                # behind reciprocal m).
                dn = psl.tile([P, 2 * NM], F32, tag="rt", name="dn")
                for m in range(NM):
                    nc.tensor.matmul(
                        dn[:, 2 * m:2 * m + 2],
                        (exp_sb[:, m * P:(m + 1) * P]),
                        (sel_sb[:]),
                        start=True,
                        stop=True,
                    )
                w_tiles = []
                for m in range(NM):
                    rec = wp.tile([P, 1], F32, tag="rec", name="rec")
                    nc.vector.reciprocal(rec[:], dn[:, 2 * m:2 * m + 1])
                    w_m = wp.tile([P, 1], F32, tag=f"w{m}", name="w_m")
                    nc.vector.tensor_tensor(
                        out=w_m[:], in0=dn[:, 2 * m + 1:2 * m + 2], in1=rec[:],
                        op=MUL,
                    )
                    w_tiles.append(w_m)
                return w_tiles

            def g_part(xt_blk, i):
                # G matmuls for stripe i + both ScalarE evictions (tanh for
                # the silu algebra, copy because p = u*g needs raw g; both
                # run during the U matmuls so g's PSUM bank frees early)
                g_ps = psgu.tile([P, TB], F32, tag="gu", name="g_ps")
                for k in range(HK):
                    nc.tensor.matmul(
                        g_ps[:],
                        (wg_sb[:, i * SW + k * P: i * SW + (k + 1) * P]),
                        (xt_blk[:, k * TB:(k + 1) * TB]),
                        start=(k == 0),
                        stop=(k == HK - 1),
                    )
                th = s1p.tile([P, TB], BF16, tag="s1", name="th")
                nc.scalar.activation(th[:], g_ps[:], AF.Tanh, scale=0.5)
                g_sb = s1p.tile([P, TB], BF16, tag="s1", name="g_sb")
                nc.scalar.activation(g_sb[:], g_ps[:], AF.Copy)
                return th, g_sb

            # block 0's router: packed matmuls ride behind the HAM warmup;
            # the sum/exp/denominator pieces are finished inside block 0's
            # stage-1 loop (hooks below) so the PE never waits on the
            # group-compaction DMA
            lt32_pro = router_pack(xt_next, warm=True)
            w_next = None
            g0_pre = None

            for b in range(NB):
                xt_blk = xt_next
                w_tiles = w_next

                # ---- stage 1: hT[i] = silu(G)*U = (G*U)*(tanh(G/2)/2+1/2),
                # [I-chunk, tok] layout. Tanh shares the Exp table set ->
                # no ACT_TABLE_LOAD thrash (the baseline's Silu forced 2
                # table swaps per block).
                # ht holds 2*silu(g)*u = p + p*tanh(g/2), p = g*u (the 1/2 is
                # folded into wd host-side). Both ScalarE evictions of g_ps
                # (tanh + copy) run during the U matmuls, so g's PSUM bank
                # frees one matmul-group early - the psgu rotation then never
                # backpressures the PE (this was ~8x 432ns PE hiccups/block).
                ht_sb = htp.tile([P, IK * TB], BF16, tag="ht")
                for i in range(IK):
                    if b == 0 and i == 4:
                        exp_pro = router_sum(lt32_pro)
                    if b == 0 and i == 6:
                        w_tiles = w_next = router_weights(exp_pro)
                    if i == 0 and g0_pre is not None:
                        # i=0's G work ran at the previous block's h0->h1
                        # boundary (see stage 2)
                        th, g_sb = g0_pre
                    else:
                        th, g_sb = g_part(xt_blk, i)
                    u_ps = psgu.tile([P, TB], F32, tag="gu", name="u_ps")
                    for k in range(HK):
                        nc.tensor.matmul(
                            u_ps[:],
                            (wu_sb[:, i * SW + k * P: i * SW + (k + 1) * P]),
                            (xt_blk[:, k * TB:(k + 1) * TB]),
                            start=(k == 0),
                            stop=(k == HK - 1),
                        )
                    p_sb = s1p.tile([P, TB], BF16, tag="s1", name="p_sb")
                    nc.vector.tensor_tensor(
                        out=p_sb[:], in0=u_ps[:], in1=g_sb[:], op=MUL,
                    )
                    t_sb = s1p.tile([P, TB], BF16, tag="s1", name="t_sb")
                    nc.vector.tensor_tensor(
                        out=t_sb[:], in0=p_sb[:], in1=th[:], op=MUL,
                    )
                    nc.vector.tensor_tensor(
                        out=ht_sb[:, i * TB:(i + 1) * TB],
                        in0=t_sb[:], in1=p_sb[:], op=ADD,
                    )

                xt_next = xt_next2
                if b + 2 < NB:
                    xt_next2 = load_xt(b + 2)

                # ---- stage 2: Y[m] [128tok, 512h] = hT^T @ Wd, scaled by w.
                # Router for block b+1 rides between/inside the h-sweeps; its
                # sum/exp/denominator pieces are spread through the h1 i-loop
                # so the PE never waits on the DVE+DMA group-compaction or
                # ScalarE's exp (w isn't needed until block b+1's stage 2).
                lt32_next = exp_next = None
                g0_pre = None
                for h in range(NH):
                    if h == 1 and b + 1 < NB:
                        # boundary filler: next block's packed router + its
                        # first G-group (neither touches y_ps banks), so the
                        # PE stays busy while h0's evictions free the y_ps
                        # slots h1's first matmuls reuse (~1.5us of sem
                        # latency otherwise)
                        lt32_next = router_pack(xt_next)
                        g0_pre = g_part(xt_next, 0)
                    y_ps = [
                        psy.tile([P, 512], F32, tag="y", name=f"y_ps{m}")
                        for m in range(NM)
                    ]
                    for i in range(IK):
                        rhs = wd_sb[:, i * H + h * 512: i * H + (h + 1) * 512]
                        for m in range(NM):
                            nc.tensor.matmul(
                                y_ps[m][:],
                                (ht_sb[:, i * TB + m * P: i * TB + (m + 1) * P]),
                                (rhs),
                                start=(i == 0),
                                stop=(i == IK - 1),
                            )
                        if h == 1 and i == 8 and lt32_next is not None:
                            exp_next = router_sum(lt32_next)
                        if h == 1 and i == 12 and exp_next is not None:
                            w_next = router_weights(exp_next)
                    # evictions split across DVE and ScalarE halves: a full
                    # [P,512] eviction is 739ns on DVE, so 4 serialized ones
                    # (3us) gated h1's first matmuls (y_ps slot reuse) for
                    # ~2.3us per block; two engines in parallel clear each
                    # bank in ~400ns right behind its stop matmul.
                    for m in range(NM):
                        y_sb = evp.tile([P, 512], F32, tag="ev", name=f"yev{h}_{m}")
                        nc.vector.tensor_scalar_mul(
                            y_sb[:, 0:256], y_ps[m][:, 0:256], w_tiles[m][:]
                        )
                        nc.scalar.activation(
                            y_sb[:, 256:512], y_ps[m][:, 256:512], AF.Copy,
                            scale=w_tiles[m][:],
                        )
                        nc.sync.dma_start(
                            out=y_d[b * TB + m * P: b * TB + (m + 1) * P,
                                    h * 512:(h + 1) * 512],
                            in_=y_sb[:],
                        )

    nc.compile()
    return nc


def kernel(x, W_router, W_gate, W_up, W_down):
    global LAST_EXEC_NS
    if "nc" not in _CACHE:
        _CACHE["nc"] = _build()
    nc = _CACHE["nc"]

    bf16 = ml_dtypes.bfloat16
    xt = np.ascontiguousarray(
        np.asarray(x, dtype=np.float32).reshape(N, H).T
    ).astype(bf16)
    wr = np.ascontiguousarray(
        np.asarray(W_router, dtype=np.float32)
        .reshape(HK, P, E).transpose(1, 0, 2).reshape(P, HK * E)
    ).astype(bf16)
    eye = np.eye(E, dtype=np.float32)
    msum = np.ascontiguousarray(np.tile(eye, (4, 1)))
    in_maps = []
    for e in range(E):
        sel = np.stack([np.ones(E, dtype=np.float32), eye[e]], axis=1)
        wg = (
            np.asarray(W_gate[e], dtype=np.float32)
            .reshape(HK, P, IK, P).transpose(1, 2, 0, 3).reshape(P, IK * SW)
        )
        wu = (
            np.asarray(W_up[e], dtype=np.float32)
            .reshape(HK, P, IK, P).transpose(1, 2, 0, 3).reshape(P, IK * SW)
        )
        # 0.5x folds the (1+tanh)/2 normalization of stage 1 into wd
        wd = (
            np.asarray(W_down[e], dtype=np.float32)
            .reshape(IK, P, H).transpose(1, 0, 2).reshape(P, IK * H)
        ) * 0.5
        in_maps.append({
            "xt": xt,
            "wg": np.ascontiguousarray(wg).astype(bf16),
            "wu": np.ascontiguousarray(wu).astype(bf16),
            "wd": np.ascontiguousarray(wd).astype(bf16),
            "wr": wr,
            "sel": np.ascontiguousarray(sel),
            "msum": msum,
        })

    res = run_bass_kernel_spmd(nc, in_maps, list(range(E)), trace=TRACE)
    LAST_EXEC_NS = res.exec_time_ns

    acc = np.zeros((N, H), dtype=np.float64)
    for r in res.results:
        acc += r["y"]
    return acc.astype(np.float32).reshape(x.shape[0], x.shape[1], H)



# revision 3
# speedup vs baseline: 1.2001x; 1.2001x over previous
"""MoE layer (dense all-experts SwiGLU + router-weighted sum) on 8 TRN2 cores.

Expert-parallel: core e holds expert e's weights (E=8). Every core sees the
full token stream x (shipped pre-transposed as xt [H, N]) and computes
  y_e = softmax(x @ W_router)[:, e] * ((silu(x@Wg_e) * (x@Wu_e)) @ Wd_e)
The host sums the 8 per-expert outputs.

~1377us vs the 1583us fp32r baseline (PE F-cycle floor ~1341us; measured
steady-state matmul cadence is the 216ns minimum = 512 cols @2.4GHz + NX
dispatch). What bought the speedup, in order of impact:
  - all matmul operands in bf16 (same 1 col/cycle PE rate as f32r, ~2e-3
    extra rel err, far under the 2e-2 gate). Halves SBUF+DMA so ALL
    weights (wg/wu/wd = 12MB) are SBUF-resident: the per-block 8MB wd
    re-stream is gone and stage 2 never waits on DMA. bf16 stationary
    weights also get FWL (LDWEIGHTS 97ns, fully hidden -> 216ns/MM vs
    233ns for f32r).
  - silu via the exp-family table: silu(g)*u = 0.5*(p + p*tanh(0.5 g)),
    p = g*u, with the 0.5 folded into wd host-side. The baseline
    alternated Silu/Exp activation tables every block: 32 ACT_TABLE_LOADs
    (~1.3us each) that stalled PSUM eviction at block starts. Tanh and
    Exp share one table set -> 1 load total, and HAM (the PE clock gate)
    stays at 8/8 the whole run.
  - column-packed router: the M=8 logit matmuls run 4-concurrent via
    tile_position col groups (2 rounds + one K=32 sum-matmul = 3 PE slots
    per block instead of 8). Group partials cross partitions via DVE
    eviction + tiny SBUF->SBUF DMAs (DVE is lane-locked; DMA can't read
    PSUM). Router pieces are spread through stage-2's h1 i-loop so the PE
    never waits on the DVE/DMA/ScalarE chain.
  - y evictions split into DVE + ScalarE halves: 4 serialized 739ns DVE
    evictions were gating h1's first matmuls (PSUM slot reuse) ~2.3us per
    block; two engines clear each bank in ~400ns.
  - startup: host pre-arranges weights into exact SBUF layouts (all DMAs
    contiguous), wg/wu stream in i-stripe order, xt is one 1MB descriptor
    per block, and ~72 tiny matmuls on wr warm the HAM clock gate while
    xt(0) streams, so real work starts ~9us in at full clock.
  - ht double-buffered across blocks; xt prefetched 2 blocks ahead.
  - boundary filler: the next block's packed router AND its first G-group
    (no y_ps banks touched) execute at the h0->h1 boundary, absorbing the
    ~300-450ns/link eviction-semaphore latency that stalled h1's first
    matmuls - steady state now shows zero PE stalls above trace noise.

Known residue (~36us over the F-cycle floor): ~7us NEFF prologue, ~12us
BSP epilogue, ~17us NX dispatch tax (216 vs 213.3ns), ~10us block-
boundary sem latencies. Runs sometimes come back ~1670us with identical
(correct) output: the chip drops the whole NeuronCore domain to 2.0GHz
(P0 power state, spacing 259ns = 216*2.4/2.0) - environmental, not
kernel-dependent.
"""
import numpy as np
import ml_dtypes

import concourse.bass as bass
import concourse.mybir as mybir
import concourse.tile as tile
from concourse import bacc
from concourse.bass_utils import run_bass_kernel_spmd

P = 128
H, I, E = 1024, 2048, 8
N = 8192       # tokens = 4 * 2048
HK = H // P    # 8 contraction chunks over H
IK = I // P    # 16 chunks over I
TB = 512       # token block
NB = N // TB   # 16 blocks
NM = TB // P   # 4 token subtiles per block
NH = H // 512  # 2 output column halves
SW = HK * P    # wg/wu stripe width: stripe i holds cols i*SW..(i+1)*SW

F32 = mybir.dt.float32
F32R = mybir.dt.float32r
BF16 = mybir.dt.bfloat16
AF = mybir.ActivationFunctionType
MUL = mybir.AluOpType.mult
ADD = mybir.AluOpType.add

# set by a driver (test.py) to profile; harness path keeps defaults
TRACE = False
LAST_EXEC_NS = None

_CACHE = {}


def _build():
    nc = bacc.Bacc("TRN2", target_bir_lowering=False, debug=False)

    xt_d = nc.dram_tensor("xt", [H, N], BF16, kind="ExternalInput").ap()
    wg_d = nc.dram_tensor("wg", [P, IK * SW], BF16, kind="ExternalInput").ap()
    wu_d = nc.dram_tensor("wu", [P, IK * SW], BF16, kind="ExternalInput").ap()
    wd_d = nc.dram_tensor("wd", [P, IK * H], BF16, kind="ExternalInput").ap()
    wr_d = nc.dram_tensor("wr", [P, HK * E], BF16, kind="ExternalInput").ap()
    sel_d = nc.dram_tensor("sel", [E, 2], F32R, kind="ExternalInput").ap()
    msum_d = nc.dram_tensor("msum", [32, E], F32R, kind="ExternalInput").ap()
    y_d = nc.dram_tensor("y", [N, H], F32, kind="ExternalOutput").ap()

    with tile.TileContext(nc) as tc:
        with (
            tc.tile_pool(name="const", bufs=1) as const,
            tc.tile_pool(name="xtp", bufs=3) as xtp,
            tc.tile_pool(name="htp", bufs=2) as htp,
            tc.tile_pool(name="s1p", bufs=8) as s1p,
            tc.tile_pool(name="evp", bufs=4) as evp,
            tc.tile_pool(name="rtp", bufs=2) as rtp,
            tc.tile_pool(name="wp", bufs=2) as wp,
            tc.tile_pool(name="psgu", bufs=4, space="PSUM") as psgu,
            tc.tile_pool(name="psy", bufs=3, space="PSUM") as psy,
            tc.tile_pool(name="psl", bufs=1, space="PSUM") as psl,
        ):
            # resident weights, in the exact host-prearranged layouts
            wg_sb = const.tile([P, IK * SW], BF16)
            wu_sb = const.tile([P, IK * SW], BF16)
            wd_sb = const.tile([P, IK * H], BF16)
            wr_sb = const.tile([P, HK * E], BF16)
            sel_sb = const.tile([E, 2], F32R)
            msum_sb = const.tile([32, E], F32R)
            # wr first (the HAM warmup matmuls need it; SWDGE's first
            # descriptor fires ~5us late, so these stay on sync), then xt(0);
            # sel/msum are only needed by block 0's mid-stage-1 router hooks
            nc.sync.dma_start(out=wr_sb[:], in_=wr_d[:])

            def load_xt(b):
                # one 1MB descriptor per 512-token block (8 separate chunk
                # DMAs cost ~650ns latency each and stretched startup ~5us)
                tok = slice(b * TB, (b + 1) * TB)
                blk = xtp.tile([P, HK * TB], BF16, tag="xt", name=f"xt{b}")
                nc.sync.dma_start(
                    out=blk[:].rearrange("p (k t) -> p k t", k=HK),
                    in_=xt_d[:, tok].rearrange("(k p) t -> p k t", p=P),
                )
                return blk

            # xt(0) split in two descriptors: the packed router's first round
            # only needs chunks 0-3, so it starts ~1.7us earlier
            xt_next = xtp.tile([P, HK * TB], BF16, tag="xt", name="xt0")
            tok0 = slice(0, TB)
            for half in range(2):
                rows = slice(half * 4 * P, (half + 1) * 4 * P)
                nc.sync.dma_start(
                    out=xt_next[:, half * 4 * TB:(half + 1) * 4 * TB]
                    .rearrange("p (k t) -> p k t", k=4),
                    in_=xt_d[rows, tok0].rearrange("(k p) t -> p k t", p=P),
                )
            # wg/wu interleaved per 256KB i-stripe: g_step(i)/u_step(i) only
            # need stripe i, so the PE starts right after xt(0) lands;
            # sel/msum slot in after stripe 0 (needed only by the i==4 hook)
            for i in range(IK):
                cols = slice(i * SW, (i + 1) * SW)
                nc.sync.dma_start(out=wg_sb[:, cols], in_=wg_d[:, cols])
                nc.sync.dma_start(out=wu_sb[:, cols], in_=wu_d[:, cols])
                if i == 0:
                    nc.sync.dma_start(out=sel_sb[:], in_=sel_d[:])
                    nc.sync.dma_start(out=msum_sb[:], in_=msum_d[:])
            # wd needed only from block-0 stage 2 (~65us in); 4x 1MB
            for q in range(4):
                cols = slice(q * 4 * H, (q + 1) * 4 * H)
                nc.sync.dma_start(out=wd_sb[:, cols], in_=wd_d[:, cols])
            xt_next2 = load_xt(1)

            def router_pack(xt_blk, warm=False):
                """Column-packed router logits: 4 concurrent M=8 matmuls per
                round (col groups 32j), 2 rounds accumulating K=2x512 over
                the 8 H-chunks. 3 PE slots/block instead of 8. The 4 group
                partials land on PSUM partitions 32j..32j+7; DVE evicts them
                lane-aligned, then tiny SBUF->SBUF DMAs compact the groups to
                partitions 0..31 for the K=32 sum-matmul in router_sum (DVE
                cannot move data across partitions; DMA cannot read PSUM)."""
                lt_ps = psl.tile([P, TB], F32, tag="rt", name="lt_ps")
                if warm:
                    # ~72 tiny matmuls on already-resident wr warm the PE's
                    # HAM clock gate (~3.4us of activity) while xt(0) still
                    # streams, so real work starts at 2.4GHz, not 1.2GHz.
                    # They write scratch rows the packed matmuls reset.
                    # (A memset-sourced variant that starts ~2.4us earlier
                    # measured ~1.5us slower overall - kept the wr version.)
                    for _ in range(72):
                        nc.tensor.matmul(
                            lt_ps[0:8, 0:64],
                            (wr_sb[:, 0:8]),
                            (wr_sb[:, 0:64]),
                            start=True,
                            stop=True,
                        )
                for r in range(2):
                    for j in range(4):
                        k = 4 * r + j
                        nc.tensor.matmul(
                            lt_ps[32 * j:32 * j + 8, :],
                            (wr_sb[:, k * E:(k + 1) * E]),
                            (xt_blk[:, k * TB:(k + 1) * TB]),
                            start=(r == 0),
                            stop=(r == 1),
                            tile_position=(0, 32 * j),
                        )
                lts = rtp.tile([P, TB], F32R, tag="lts", name="lts")
                for j in range(4):
                    nc.vector.tensor_copy(
                        out=lts[32 * j:32 * j + 8, :],
                        in_=lt_ps[32 * j:32 * j + 8, :],
                    )
                lt32 = rtp.tile([32, TB], F32R, tag="lt32", name="lt32")
                for j in range(4):
                    nc.gpsimd.dma_start(
                        out=lt32[8 * j:8 * j + 8, :],
                        in_=lts[32 * j:32 * j + 8, :],
                    )
                return lt32

            def router_sum(lt32):
                lt2 = psl.tile([E, TB], F32, tag="rt", name="lt2")
                nc.tensor.matmul(
                    lt2[:], (msum_sb[:]), (lt32[:]), start=True, stop=True
                )
                exp_sb = rtp.tile([E, TB], F32R, tag="exp", name="exp_sb")
                nc.scalar.activation(exp_sb[:], lt2[:], AF.Exp)
                return exp_sb

            def router_weights(exp_sb):
                # w[tok] = exp_e / sum_e' exp_e' via per-subtile transpose-mm.
                # All 4 [denom|numer] matmuls land in one PSUM tile so they
                # issue back-to-back on the PE with no DVE dependency between
                # them (separate bufs=1 tiles would serialize matmul m+1
                # behind reciprocal m).
                dn = psl.tile([P, 2 * NM], F32, tag="rt", name="dn")
                for m in range(NM):
                    nc.tensor.matmul(
                        dn[:, 2 * m:2 * m + 2],
                        (exp_sb[:, m * P:(m + 1) * P]),
                        (sel_sb[:]),
                        start=True,
                        stop=True,
                    )
                w_tiles = []
                for m in range(NM):
                    rec = wp.tile([P, 1], F32, tag="rec", name="rec")
                    nc.vector.reciprocal(rec[:], dn[:, 2 * m:2 * m + 1])
                    w_m = wp.tile([P, 1], F32, tag=f"w{m}", name="w_m")
                    nc.vector.tensor_tensor(
                        out=w_m[:], in0=dn[:, 2 * m + 1:2 * m + 2], in1=rec[:],
                        op=MUL,
                    )
                    w_tiles.append(w_m)
                return w_tiles

            def g_part(xt_blk, i):
                # G matmuls for stripe i + both ScalarE evictions (tanh for
                # the silu algebra, copy because p = u*g needs raw g; both
                # run during the U matmuls so g's PSUM bank frees early)
                g_ps = psgu.tile([P, TB], F32, tag="gu", name="g_ps")
                for k in range(HK):
                    nc.tensor.matmul(
                        g_ps[:],
                        (wg_sb[:, i * SW + k * P: i * SW + (k + 1) * P]),
                        (xt_blk[:, k * TB:(k + 1) * TB]),
                        start=(k == 0),
                        stop=(k == HK - 1),
                    )
                th = s1p.tile([P, TB], BF16, tag="s1", name="th")
                nc.scalar.activation(th[:], g_ps[:], AF.Tanh, scale=0.5)
                g_sb = s1p.tile([P, TB], BF16, tag="s1", name="g_sb")
                nc.scalar.activation(g_sb[:], g_ps[:], AF.Copy)
                return th, g_sb

            # block 0's router: packed matmuls ride behind the HAM warmup;
            # the sum/exp/denominator pieces are finished inside block 0's
            # stage-1 loop (hooks below) so the PE never waits on the
            # group-compaction DMA
            lt32_pro = router_pack(xt_next, warm=True)
            w_next = None
            g0_pre = None

            for b in range(NB):
                xt_blk = xt_next
                w_tiles = w_next

                # ---- stage 1: hT[i] = silu(G)*U = (G*U)*(tanh(G/2)/2+1/2),
                # [I-chunk, tok] layout. Tanh shares the Exp table set ->
                # no ACT_TABLE_LOAD thrash (the baseline's Silu forced 2
                # table swaps per block).
                # ht holds 2*silu(g)*u = p + p*tanh(g/2), p = g*u (the 1/2 is
                # folded into wd host-side). Both ScalarE evictions of g_ps
                # (tanh + copy) run during the U matmuls, so g's PSUM bank
                # frees one matmul-group early - the psgu rotation then never
                # backpressures the PE (this was ~8x 432ns PE hiccups/block).
                ht_sb = htp.tile([P, IK * TB], BF16, tag="ht")
                for i in range(IK):
                    if b == 0 and i == 4:
                        exp_pro = router_sum(lt32_pro)
                    if b == 0 and i == 6:
                        w_tiles = w_next = router_weights(exp_pro)
                    if i == 0 and g0_pre is not None:
                        # i=0's G work ran at the previous block's h0->h1
                        # boundary (see stage 2)
                        th, g_sb = g0_pre
                    else:
                        th, g_sb = g_part(xt_blk, i)
                    u_ps = psgu.tile([P, TB], F32, tag="gu", name="u_ps")
                    for k in range(HK):
                        nc.tensor.matmul(
                            u_ps[:],
                            (wu_sb[:, i * SW + k * P: i * SW + (k + 1) * P]),
                            (xt_blk[:, k * TB:(k + 1) * TB]),
                            start=(k == 0),
                            stop=(k == HK - 1),
                        )
                    p_sb = s1p.tile([P, TB], BF16, tag="s1", name="p_sb")
                    nc.vector.tensor_tensor(
                        out=p_sb[:], in0=u_ps[:], in1=g_sb[:], op=MUL,
                    )
                    t_sb = s1p.tile([P, TB], BF16, tag="s1", name="t_sb")
                    nc.vector.tensor_tensor(
                        out=t_sb[:], in0=p_sb[:], in1=th[:], op=MUL,
                    )
                    nc.vector.tensor_tensor(
                        out=ht_sb[:, i * TB:(i + 1) * TB],
                        in0=t_sb[:], in1=p_sb[:], op=ADD,
                    )

                xt_next = xt_next2
                if b + 2 < NB:
                    xt_next2 = load_xt(b + 2)

                # ---- stage 2: m-outer, per-m h0 then h1 sweeps.
                # Y[m,h] [128tok, 512h] = hT^T @ Wd, scaled by w. Each sweep
                # accumulates one PSUM bank over the full i range; its
                # eviction overlaps the next sweep (>=3.4us of slack), so
                # psy needs only 3 banks and psgu gets a 4th - that extra
                # slot kills the recurring ~63ns stage-1 G-start stalls
                # (sem-hop latency on the 3-deep psgu rotation). The old
                # h-outer form held 4 y banks live and bunched 4 evictions
                # at the h0->h1 boundary (filler-absorbed mid-run but fully
                # exposed on the last block's tail). Per m both halves land
                # in one [P,1024] evp tile -> one contiguous row-chunk DMA.
                # Router for block b+1 rides between/inside the sweeps.
                lt32_next = exp_next = None
                g0_pre = None
                for m in range(NM):
                    ev = evp.tile([P, 1024], F32, tag="ev", name=f"yev{m}")
                    for h in range(NH):
                        if m == 2 and h == 0 and b + 1 < NB:
                            # filler: next block's packed router + its first
                            # G-group keep the PE fed while this block's
                            # eviction/DMA chains drain (neither touches psy)
                            lt32_next = router_pack(xt_next)
                            g0_pre = g_part(xt_next, 0)
                        y_ps = psy.tile([P, 512], F32, tag="y", name=f"y{m}{h}")
                        for i in range(IK):
                            nc.tensor.matmul(
                                y_ps[:],
                                (ht_sb[:, i * TB + m * P: i * TB + (m + 1) * P]),
                                (wd_sb[:, i * H + h * 512: i * H + (h + 1) * 512]),
                                start=(i == 0),
                                stop=(i == IK - 1),
                            )
                            if m == 3 and h == 0 and i == 4 and lt32_next is not None:
                                exp_next = router_sum(lt32_next)
                            if m == 3 and h == 1 and i == 4 and exp_next is not None:
                                w_next = router_weights(exp_next)
                        # DVE takes h0, ScalarE h1: balanced, uniform, and
                        # each eviction has a whole sweep before its bank is
                        # reused by the psy rotation
                        if h == 0:
                            nc.vector.tensor_scalar_mul(
                                ev[:, 0:512], y_ps[:], w_tiles[m][:]
                            )
                        else:
                            nc.scalar.activation(
                                ev[:, 512:1024], y_ps[:], AF.Copy,
                                scale=w_tiles[m][:],
                            )
                    nc.sync.dma_start(
                        out=y_d[b * TB + m * P: b * TB + (m + 1) * P, :],
                        in_=ev[:],
                    )

    nc.compile()
    return nc


def kernel(x, W_router, W_gate, W_up, W_down):
    global LAST_EXEC_NS
    if "nc" not in _CACHE:
        _CACHE["nc"] = _build()
    nc = _CACHE["nc"]

    bf16 = ml_dtypes.bfloat16
    xt = np.ascontiguousarray(
        np.asarray(x, dtype=np.float32).reshape(N, H).T
    ).astype(bf16)
    wr = np.ascontiguousarray(
        np.asarray(W_router, dtype=np.float32)
        .reshape(HK, P, E).transpose(1, 0, 2).reshape(P, HK * E)
    ).astype(bf16)
    eye = np.eye(E, dtype=np.float32)
    msum = np.ascontiguousarray(np.tile(eye, (4, 1)))
    in_maps = []
    for e in range(E):
        sel = np.stack([np.ones(E, dtype=np.float32), eye[e]], axis=1)
        wg = (
            np.asarray(W_gate[e], dtype=np.float32)
            .reshape(HK, P, IK, P).transpose(1, 2, 0, 3).reshape(P, IK * SW)
        )
        wu = (
            np.asarray(W_up[e], dtype=np.float32)
            .reshape(HK, P, IK, P).transpose(1, 2, 0, 3).reshape(P, IK * SW)
        )
        # 0.5x folds the (1+tanh)/2 normalization of stage 1 into wd
        wd = (
            np.asarray(W_down[e], dtype=np.float32)
            .reshape(IK, P, H).transpose(1, 0, 2).reshape(P, IK * H)
        ) * 0.5
        in_maps.append({
            "xt": xt,
            "wg": np.ascontiguousarray(wg).astype(bf16),
            "wu": np.ascontiguousarray(wu).astype(bf16),
            "wd": np.ascontiguousarray(wd).astype(bf16),
            "wr": wr,
            "sel": np.ascontiguousarray(sel),
            "msum": msum,
        })

    res = run_bass_kernel_spmd(nc, in_maps, list(range(E)), trace=TRACE)
    LAST_EXEC_NS = res.exec_time_ns

    acc = np.zeros((N, H), dtype=np.float64)
    for r in res.results:
        acc += r["y"]
    return acc.astype(np.float32).reshape(x.shape[0], x.shape[1], H)



# revision 11
# speedup vs baseline: 1.2056x; 1.0046x over previous
"""MoE layer (dense all-experts SwiGLU + router-weighted sum) on 8 TRN2 cores.

Expert-parallel: core e holds expert e's weights (E=8). Every core sees the
full token stream x (shipped pre-transposed as xt [H, N]) and computes
  y_e = softmax(x @ W_router)[:, e] * ((silu(x@Wg_e) * (x@Wu_e)) @ Wd_e)
The host sums the 8 per-expert outputs.

~1377us vs the 1583us fp32r baseline (PE F-cycle floor ~1341us; measured
steady-state matmul cadence is the 216ns minimum = 512 cols @2.4GHz + NX
dispatch). What bought the speedup, in order of impact:
  - all matmul operands in bf16 (same 1 col/cycle PE rate as f32r, ~2e-3
    extra rel err, far under the 2e-2 gate). Halves SBUF+DMA so ALL
    weights (wg/wu/wd = 12MB) are SBUF-resident: the per-block 8MB wd
    re-stream is gone and stage 2 never waits on DMA. bf16 stationary
    weights also get FWL (LDWEIGHTS 97ns, fully hidden -> 216ns/MM vs
    233ns for f32r).
  - silu via the exp-family table: silu(g)*u = 0.5*(p + p*tanh(0.5 g)),
    p = g*u, with the 0.5 folded into wd host-side. The baseline
    alternated Silu/Exp activation tables every block: 32 ACT_TABLE_LOADs
    (~1.3us each) that stalled PSUM eviction at block starts. Tanh and
    Exp share one table set -> 1 load total, and HAM (the PE clock gate)
    stays at 8/8 the whole run.
  - column-packed router: the M=8 logit matmuls run 4-concurrent via
    tile_position col groups (2 rounds + one K=32 sum-matmul = 3 PE slots
    per block instead of 8). Group partials cross partitions via DVE
    eviction + tiny SBUF->SBUF DMAs (DVE is lane-locked; DMA can't read
    PSUM). Router pieces are spread through stage-2's h1 i-loop so the PE
    never waits on the DVE/DMA/ScalarE chain.
  - y evictions split into DVE + ScalarE halves: 4 serialized 739ns DVE
    evictions were gating h1's first matmuls (PSUM slot reuse) ~2.3us per
    block; two engines clear each bank in ~400ns.
  - startup: host pre-arranges weights into exact SBUF layouts (all DMAs
    contiguous), wg/wu stream in i-stripe order, xt is one 1MB descriptor
    per block, and ~72 tiny matmuls on wr warm the HAM clock gate while
    xt(0) streams, so real work starts ~9us in at full clock.
  - ht double-buffered across blocks; xt prefetched 2 blocks ahead.
  - boundary filler: the next block's packed router AND its first G-group
    (no y_ps banks touched) execute at the h0->h1 boundary, absorbing the
    ~300-450ns/link eviction-semaphore latency that stalled h1's first
    matmuls - steady state now shows zero PE stalls above trace noise.

Known residue (~36us over the F-cycle floor): ~7us NEFF prologue, ~12us
BSP epilogue, ~17us NX dispatch tax (216 vs 213.3ns), ~10us block-
boundary sem latencies. Runs sometimes come back ~1670us with identical
(correct) output: the chip drops the whole NeuronCore domain to 2.0GHz
(P0 power state, spacing 259ns = 216*2.4/2.0) - environmental, not
kernel-dependent.
"""
import numpy as np
import ml_dtypes

import concourse.bass as bass
import concourse.mybir as mybir
import concourse.tile as tile
from concourse import bacc
from concourse.bass_utils import run_bass_kernel_spmd

P = 128
H, I, E = 1024, 2048, 8
N = 8192       # tokens = 4 * 2048
HK = H // P    # 8 contraction chunks over H
IK = I // P    # 16 chunks over I
TB = 512       # token block
NB = N // TB   # 16 blocks
NM = TB // P   # 4 token subtiles per block
NH = H // 512  # 2 output column halves
SW = HK * P    # wg/wu stripe width: stripe i holds cols i*SW..(i+1)*SW

F32 = mybir.dt.float32
F32R = mybir.dt.float32r
BF16 = mybir.dt.bfloat16
AF = mybir.ActivationFunctionType
MUL = mybir.AluOpType.mult
ADD = mybir.AluOpType.add

# set by a driver (test.py) to profile; harness path keeps defaults
TRACE = False
LAST_EXEC_NS = None

_CACHE = {}


def _build():
    nc = bacc.Bacc("TRN2", target_bir_lowering=False, debug=False)

    # xt shipped block-major: block b is 1MB contiguous in the exact SBUF
    # layout [P, HK*TB] (the old [H, N] layout made each block a 1KB-grain
    # strided gather - ~98GB/s effective, xt(0) landed ~14us in and gated
    # the whole startup)
    xt_d = nc.dram_tensor("xt", [NB, P, HK * TB], BF16, kind="ExternalInput").ap()
    wg_d = nc.dram_tensor("wg", [P, IK * SW], BF16, kind="ExternalInput").ap()
    wu_d = nc.dram_tensor("wu", [P, IK * SW], BF16, kind="ExternalInput").ap()
    wd_d = nc.dram_tensor("wd", [P, IK * H], BF16, kind="ExternalInput").ap()
    wr_d = nc.dram_tensor("wr", [P, HK * E], BF16, kind="ExternalInput").ap()
    sel_d = nc.dram_tensor("sel", [E, 2], F32R, kind="ExternalInput").ap()
    y_d = nc.dram_tensor("y", [N, H], F32, kind="ExternalOutput").ap()

    with tile.TileContext(nc) as tc:
        with (
            tc.tile_pool(name="const", bufs=1) as const,
            tc.tile_pool(name="xtp", bufs=3) as xtp,
            tc.tile_pool(name="htp", bufs=2) as htp,
            tc.tile_pool(name="s1p", bufs=8) as s1p,
            tc.tile_pool(name="evp", bufs=4) as evp,
            tc.tile_pool(name="rtp", bufs=2) as rtp,
            tc.tile_pool(name="wp", bufs=2) as wp,
            tc.tile_pool(name="psgu", bufs=4, space="PSUM") as psgu,
            tc.tile_pool(name="psy", bufs=3, space="PSUM") as psy,
            tc.tile_pool(name="psl", bufs=1, space="PSUM") as psl,
        ):
            # resident weights, in the exact host-prearranged layouts
            wg_sb = const.tile([P, IK * SW], BF16)
            wu_sb = const.tile([P, IK * SW], BF16)
            wd_sb = const.tile([P, IK * H], BF16)
            wr_sb = const.tile([P, HK * E], BF16)
            sel_sb = const.tile([E, 2], F32R)
            # wr first (the HAM warmup matmuls need it; SWDGE's first
            # descriptor fires ~5us late, so these stay on sync), then xt(0);
            # sel is only needed by block 0's mid-stage-1 router hooks
            nc.sync.dma_start(out=wr_sb[:], in_=wr_d[:])

            def load_xt(b):
                # one contiguous 1MB descriptor per 512-token block
                blk = xtp.tile([P, HK * TB], BF16, tag="xt", name=f"xt{b}")
                nc.sync.dma_start(out=blk[:], in_=xt_d[b])
                return blk

            # xt(0) split in two descriptors: the packed router's first round
            # only needs chunks 0-3, so it starts earlier
            xt_next = xtp.tile([P, HK * TB], BF16, tag="xt", name="xt0")
            for half in range(2):
                cols = slice(half * 4 * TB, (half + 1) * 4 * TB)
                nc.sync.dma_start(out=xt_next[:, cols], in_=xt_d[0][:, cols])
            # wg/wu interleaved per 256KB i-stripe: g_step(i)/u_step(i) only
            # need stripe i, so the PE starts right after xt(0) lands;
            # sel slots in after stripe 0 (needed only by the i==6 hook)
            for i in range(IK):
                cols = slice(i * SW, (i + 1) * SW)
                nc.sync.dma_start(out=wg_sb[:, cols], in_=wg_d[:, cols])
                nc.sync.dma_start(out=wu_sb[:, cols], in_=wu_d[:, cols])
                if i == 0:
                    nc.sync.dma_start(out=sel_sb[:], in_=sel_d[:])
            # wd needed only from block-0 stage 2 (~65us in); 4x 1MB
            for q in range(4):
                cols = slice(q * 4 * H, (q + 1) * 4 * H)
                nc.sync.dma_start(out=wd_sb[:, cols], in_=wd_d[:, cols])
            xt_next2 = load_xt(1)

            def router_pack(xt_blk, warm=False):
                """Column-packed router logits: 4 concurrent M=8 matmuls per
                round (col groups 32j), 2 rounds accumulating K=2x512 over
                the 8 H-chunks. 3 PE slots/block instead of 8. The 4 group
                partials land on PSUM partitions 32j..32j+7; DVE evicts them
                lane-aligned, then tiny SBUF->SBUF DMAs compact the groups to
                partitions 0..31 for the K=32 sum-matmul in router_sum (DVE
                cannot move data across partitions; DMA cannot read PSUM)."""
                lt_ps = psl.tile([P, TB], F32, tag="rt", name="lt_ps")
                if warm:
                    # ~72 tiny matmuls on already-resident wr warm the PE's
                    # HAM clock gate (~3.4us of activity) while xt(0) still
                    # streams, so real work starts at 2.4GHz, not 1.2GHz.
                    # They write scratch rows the packed matmuls reset.
                    # (A memset-sourced variant that starts ~2.4us earlier
                    # measured ~1.5us slower overall - kept the wr version.)
                    for _ in range(72):
                        nc.tensor.matmul(
                            lt_ps[0:8, 0:64],
                            (wr_sb[:, 0:8]),
                            (wr_sb[:, 0:64]),
                            start=True,
                            stop=True,
                        )
                for r in range(2):
                    for j in range(4):
                        k = 4 * r + j
                        nc.tensor.matmul(
                            lt_ps[32 * j:32 * j + 8, :],
                            (wr_sb[:, k * E:(k + 1) * E]),
                            (xt_blk[:, k * TB:(k + 1) * TB]),
                            start=(r == 0),
                            stop=(r == 1),
                            tile_position=(0, 32 * j),
                        )
                lts = rtp.tile([P, TB], F32R, tag="lts", name="lts")
                for j in range(4):
                    nc.vector.tensor_copy(
                        out=lts[32 * j:32 * j + 8, :],
                        in_=lt_ps[32 * j:32 * j + 8, :],
                    )
                # compact the 4 group partials to partitions 0..7 stacked
                # along the free dim, so the group-sum can run as plain DVE
                # adds (same partitions) instead of the old K=32 sum-matmul -
                # that was one more 213ns PE slot per block
                lt32 = rtp.tile([E, 4 * TB], F32R, tag="lt32", name="lt32", bufs=1)
                for j in range(4):
                    nc.gpsimd.dma_start(
                        out=lt32[:, j * TB:(j + 1) * TB],
                        in_=lts[32 * j:32 * j + 8, :],
                    )
                return lt32

            def router_sum(lt32):
                s0 = rtp.tile([E, TB], F32R, tag="s0", name="s0", bufs=1)
                nc.vector.tensor_tensor(
                    out=s0[:], in0=lt32[:, 0:TB], in1=lt32[:, TB:2 * TB], op=ADD,
                )
                s1 = rtp.tile([E, TB], F32R, tag="s1r", name="s1r", bufs=1)
                nc.vector.tensor_tensor(
                    out=s1[:], in0=lt32[:, 2 * TB:3 * TB], in1=lt32[:, 3 * TB:4 * TB],
                    op=ADD,
                )
                lt_sum = rtp.tile([E, TB], F32R, tag="ltsum", name="lt_sum", bufs=1)
                nc.vector.tensor_tensor(
                    out=lt_sum[:], in0=s0[:], in1=s1[:], op=ADD,
                )
                exp_sb = rtp.tile([E, TB], F32R, tag="exp", name="exp_sb")
                nc.scalar.activation(exp_sb[:], lt_sum[:], AF.Exp)
                return exp_sb

            def router_weights(exp_sb):
                # w[tok] = exp_e / sum_e' exp_e' via per-subtile transpose-mm.
                # All 4 [denom|numer] matmuls land in one PSUM tile so they
                # issue back-to-back on the PE with no DVE dependency between
                # them (separate bufs=1 tiles would serialize matmul m+1
                # behind reciprocal m).
                dn = psl.tile([P, 2 * NM], F32, tag="rt", name="dn")
                for m in range(NM):
                    nc.tensor.matmul(
                        dn[:, 2 * m:2 * m + 2],
                        (exp_sb[:, m * P:(m + 1) * P]),
                        (sel_sb[:]),
                        start=True,
                        stop=True,
                    )
                w_tiles = []
                for m in range(NM):
                    rec = wp.tile([P, 1], F32, tag="rec", name="rec")
                    nc.vector.reciprocal(rec[:], dn[:, 2 * m:2 * m + 1])
                    w_m = wp.tile([P, 1], F32, tag=f"w{m}", name="w_m")
                    nc.vector.tensor_tensor(
                        out=w_m[:], in0=dn[:, 2 * m + 1:2 * m + 2], in1=rec[:],
                        op=MUL,
                    )
                    w_tiles.append(w_m)
                return w_tiles

            def g_part(xt_blk, i):
                # G matmuls for stripe i + both ScalarE evictions (tanh for
                # the silu algebra, copy because p = u*g needs raw g; both
                # run during the U matmuls so g's PSUM bank frees early)
                g_ps = psgu.tile([P, TB], F32, tag="gu", name="g_ps")
                for k in range(HK):
                    nc.tensor.matmul(
                        g_ps[:],
                        (wg_sb[:, i * SW + k * P: i * SW + (k + 1) * P]),
                        (xt_blk[:, k * TB:(k + 1) * TB]),
                        start=(k == 0),
                        stop=(k == HK - 1),
                    )
                th = s1p.tile([P, TB], BF16, tag="s1", name="th")
                nc.scalar.activation(th[:], g_ps[:], AF.Tanh, scale=0.5)
                g_sb = s1p.tile([P, TB], BF16, tag="s1", name="g_sb")
                nc.scalar.activation(g_sb[:], g_ps[:], AF.Copy)
                return th, g_sb

            # block 0's router: packed matmuls ride behind the HAM warmup;
            # the sum/exp/denominator pieces are finished inside block 0's
            # stage-1 loop (hooks below) so the PE never waits on the
            # group-compaction DMA
            lt32_pro = router_pack(xt_next, warm=True)
            w_next = None
            g0_pre = None

            for b in range(NB):
                xt_blk = xt_next
                w_tiles = w_next

                # ---- stage 1: hT[i] = silu(G)*U = (G*U)*(tanh(G/2)/2+1/2),
                # [I-chunk, tok] layout. Tanh shares the Exp table set ->
                # no ACT_TABLE_LOAD thrash (the baseline's Silu forced 2
                # table swaps per block).
                # ht holds 2*silu(g)*u = p + p*tanh(g/2), p = g*u (the 1/2 is
                # folded into wd host-side). Both ScalarE evictions of g_ps
                # (tanh + copy) run during the U matmuls, so g's PSUM bank
                # frees one matmul-group early - the psgu rotation then never
                # backpressures the PE (this was ~8x 432ns PE hiccups/block).
                ht_sb = htp.tile([P, IK * TB], BF16, tag="ht")
                for i in range(IK):
                    if b == 0 and i == 4:
                        exp_pro = router_sum(lt32_pro)
                    if b == 0 and i == 6:
                        w_tiles = w_next = router_weights(exp_pro)
                    if i == 0 and g0_pre is not None:
                        # i=0's G work ran at the previous block's h0->h1
                        # boundary (see stage 2)
                        th, g_sb = g0_pre
                    else:
                        th, g_sb = g_part(xt_blk, i)
                    u_ps = psgu.tile([P, TB], F32, tag="gu", name="u_ps")
                    for k in range(HK):
                        nc.tensor.matmul(
                            u_ps[:],
                            (wu_sb[:, i * SW + k * P: i * SW + (k + 1) * P]),
                            (xt_blk[:, k * TB:(k + 1) * TB]),
                            start=(k == 0),
                            stop=(k == HK - 1),
                        )
                    p_sb = s1p.tile([P, TB], BF16, tag="s1", name="p_sb")
                    nc.vector.tensor_tensor(
                        out=p_sb[:], in0=u_ps[:], in1=g_sb[:], op=MUL,
                    )
                    t_sb = s1p.tile([P, TB], BF16, tag="s1", name="t_sb")
                    nc.vector.tensor_tensor(
                        out=t_sb[:], in0=p_sb[:], in1=th[:], op=MUL,
                    )
                    nc.vector.tensor_tensor(
                        out=ht_sb[:, i * TB:(i + 1) * TB],
                        in0=t_sb[:], in1=p_sb[:], op=ADD,
                    )

                xt_next = xt_next2
                if b + 2 < NB:
                    xt_next2 = load_xt(b + 2)

                # ---- stage 2: m-outer, per-m h0 then h1 sweeps.
                # Y[m,h] [128tok, 512h] = hT^T @ Wd, scaled by w. Each sweep
                # accumulates one PSUM bank over the full i range; its
                # eviction overlaps the next sweep (>=3.4us of slack), so
                # psy needs only 3 banks and psgu gets a 4th - that extra
                # slot kills the recurring ~63ns stage-1 G-start stalls
                # (sem-hop latency on the 3-deep psgu rotation). The old
                # h-outer form held 4 y banks live and bunched 4 evictions
                # at the h0->h1 boundary (filler-absorbed mid-run but fully
                # exposed on the last block's tail). Per m both halves land
                # in one [P,1024] evp tile -> one contiguous row-chunk DMA.
                # Router for block b+1 rides between/inside the sweeps.
                lt32_next = exp_next = None
                g0_pre = None
                for m in range(NM):
                    ev = evp.tile([P, 1024], F32, tag="ev", name=f"yev{m}")
                    for h in range(NH):
                        if m == 2 and h == 0 and b + 1 < NB:
                            # filler: next block's packed router + its first
                            # G-group keep the PE fed while this block's
                            # eviction/DMA chains drain (neither touches psy)
                            lt32_next = router_pack(xt_next)
                            g0_pre = g_part(xt_next, 0)
                        y_ps = psy.tile([P, 512], F32, tag="y", name=f"y{m}{h}")
                        for i in range(IK):
                            nc.tensor.matmul(
                                y_ps[:],
                                (ht_sb[:, i * TB + m * P: i * TB + (m + 1) * P]),
                                (wd_sb[:, i * H + h * 512: i * H + (h + 1) * 512]),
                                start=(i == 0),
                                stop=(i == IK - 1),
                            )
                            if m == 3 and h == 0 and i == 4 and lt32_next is not None:
                                exp_next = router_sum(lt32_next)
                            if m == 3 and h == 1 and i == 4 and exp_next is not None:
                                w_next = router_weights(exp_next)
                        # DVE takes h0, ScalarE h1: balanced, uniform, and
                        # each eviction has a whole sweep before its bank is
                        # reused by the psy rotation
                        if h == 0:
                            nc.vector.tensor_scalar_mul(
                                ev[:, 0:512], y_ps[:], w_tiles[m][:]
                            )
                            if b == NB - 1:
                                # last block: nothing overlaps the final
                                # drain, so ship each half as soon as its
                                # eviction lands (halves the tail DMA)
                                nc.sync.dma_start(
                                    out=y_d[b * TB + m * P: b * TB + (m + 1) * P,
                                            0:512],
                                    in_=ev[:, 0:512],
                                )
                        else:
                            nc.scalar.activation(
                                ev[:, 512:1024], y_ps[:], AF.Copy,
                                scale=w_tiles[m][:],
                            )
                    if b == NB - 1:
                        nc.sync.dma_start(
                            out=y_d[b * TB + m * P: b * TB + (m + 1) * P, 512:1024],
                            in_=ev[:, 512:1024],
                        )
                    else:
                        nc.sync.dma_start(
                            out=y_d[b * TB + m * P: b * TB + (m + 1) * P, :],
                            in_=ev[:],
                        )

    nc.compile()
    return nc


def kernel(x, W_router, W_gate, W_up, W_down):
    global LAST_EXEC_NS
    if "nc" not in _CACHE:
        _CACHE["nc"] = _build()
    nc = _CACHE["nc"]

    bf16 = ml_dtypes.bfloat16
    # block-major xt: block b contiguous in the exact SBUF tile layout
    # [P, HK*TB] (partition p holds chunk-k cols for tokens of block b)
    xt = np.ascontiguousarray(
        np.asarray(x, dtype=np.float32).reshape(N, H).T      # [H, N]
        .reshape(HK, P, NB, TB).transpose(2, 1, 0, 3)        # [NB, P, HK, TB]
        .reshape(NB, P, HK * TB)
    ).astype(bf16)
    wr = np.ascontiguousarray(
        np.asarray(W_router, dtype=np.float32)
        .reshape(HK, P, E).transpose(1, 0, 2).reshape(P, HK * E)
    ).astype(bf16)
    eye = np.eye(E, dtype=np.float32)
    in_maps = []
    for e in range(E):
        sel = np.stack([np.ones(E, dtype=np.float32), eye[e]], axis=1)
        wg = (
            np.asarray(W_gate[e], dtype=np.float32)
            .reshape(HK, P, IK, P).transpose(1, 2, 0, 3).reshape(P, IK * SW)
        )
        wu = (
            np.asarray(W_up[e], dtype=np.float32)
            .reshape(HK, P, IK, P).transpose(1, 2, 0, 3).reshape(P, IK * SW)
        )
        # 0.5x folds the (1+tanh)/2 normalization of stage 1 into wd
        wd = (
            np.asarray(W_down[e], dtype=np.float32)
            .reshape(IK, P, H).transpose(1, 0, 2).reshape(P, IK * H)
        ) * 0.5
        in_maps.append({
            "xt": xt,
            "wg": np.ascontiguousarray(wg).astype(bf16),
            "wu": np.ascontiguousarray(wu).astype(bf16),
            "wd": np.ascontiguousarray(wd).astype(bf16),
            "wr": wr,
            "sel": np.ascontiguousarray(sel),
        })

    res = run_bass_kernel_spmd(nc, in_maps, list(range(E)), trace=TRACE)
    LAST_EXEC_NS = res.exec_time_ns

    acc = np.zeros((N, H), dtype=np.float64)
    for r in res.results:
        acc += r["y"]
    return acc.astype(np.float32).reshape(x.shape[0], x.shape[1], H)

